# revision 1
# baseline (speedup 1.0000x reference)
"""Trainium2 Bass kernel for 16-head causal MHA (B=2, T=2048, C=1024, H=16, D=64).

Sharding: 8 cores = 2 batch groups x 4 head groups (4 heads each).
Each core computes, for its batch b and heads hg*4..hg*4+3:
  Q^T,K^T = projections kept transposed [dims, tokens] (fp32r matmuls)
  V       = projection transposed back to [tokens, dims] via PE transpose,
            augmented with a ones column per head (denominator trick)
  S^T     = K Q^T per (ts-tile, tq-chunk), causal-masked on the diagonal
            128-block only (fully-masked columns skipped), exp'd (scale
            folded into Wq on host)
  O^T_aug = V_aug^T P^T accumulated over ts tiles; row 64 is the softmax
            denominator; normalized via GPSIMD partition_broadcast + DVE
  Y_part  = O^T.T @ Wo_slice^T, interleaved per chunk  [2048, 1024]
Host sums the 4 head-group partials per batch and adds bo.
"""

import sys

sys.path.insert(0, "/opt/trn_rl_repo")

import numpy as np

import concourse.bass as bass
from concourse import bacc
import concourse.mybir as mybir
from concourse.tile import TileContext
from concourse.bass_utils import run_bass_kernel_spmd
from concourse.masks import make_identity

F32 = mybir.dt.float32
F32R = mybir.dt.float32r
EXP = mybir.ActivationFunctionType.Exp

B, T, C, H, D = 2, 2048, 1024, 16, 64
NHPC = 4          # heads per core
DH = NHPC * D     # 256 head dims per core
P = 128           # partitions
CH = 512          # token chunk (matmul moving dim)
NCHUNK = T // CH  # 4
NTT = T // P      # 16 token tiles
NCT = C // P      # 8 contraction tiles over C
NEG = -1.0e10


def build_nc(loop_reps=None, stages=3, no_mask=False, no_norm=False,
             interleave_out=True, skip_v=False, skip_proj_copies=False):
    nc = bacc.Bacc()
    xT_d = nc.declare_dram_parameter("xT", [C, T], F32R, isOutput=False)
    wqkv_d = nc.declare_dram_parameter("Wqkv", [C, 3 * DH], F32R, isOutput=False)
    wot_d = nc.declare_dram_parameter("WoT", [DH, C], F32R, isOutput=False)
    y_d = nc.declare_dram_parameter("Y", [T, C], F32, isOutput=True)

    xT = xT_d[:, :]
    wqkv = wqkv_d[:, :]
    wot = wot_d[:, :]
    y = y_d[:, :]

    with TileContext(nc) as tc:
        with (
            tc.tile_pool(name="const", bufs=1) as const,
            tc.tile_pool(name="persist", bufs=1) as persist,
        ):
            # ---- constants ----
            ones_f32 = const.tile([P, 1], F32)
            nc.gpsimd.memset(ones_f32[:], 1.0)
            ones_row = const.tile([1, D], F32)
            nc.gpsimd.memset(ones_row[:], 1.0)
            ones_col = const.tile([1, D], F32R)
            nc.vector.tensor_copy(ones_col[:], ones_row[:])
            # triangular mask for the diagonal 128x128 block (both halves):
            # mask[r, (half, j)] = 0 if r <= j else -1e10
            mask128 = const.tile([P, 2, P], F32, name="mask128")
            nc.gpsimd.memset(mask128[:], 0.0)
            nc.gpsimd.affine_select(
                out=mask128[:],
                in_=mask128[:],
                compare_op=mybir.AluOpType.is_ge,
                fill=NEG,
                base=0,
                pattern=[[0, 2], [1, P]],
                channel_multiplier=-1,
            )

            # ---- persistent tensors ----
            wq_t = [persist.tile([P, 3 * DH], F32R, name=f"wqkv{c}")
                    for c in range(NCT)]
            wot_t = [persist.tile([P, C], F32R, name=f"wot{k}") for k in range(2)]
            # Q^T/K^T [dims, tokens]; pair p holds heads (2p, 2p+1)
            qt_t = [persist.tile([P, T], F32R, name=f"qt{p}") for p in range(2)]
            kt_t = [persist.tile([P, T], F32R, name=f"kt{p}") for p in range(2)]
            # V augmented with a ones column per head: [tokens, 4*65]
            vaug_t = [persist.tile([P, NHPC * (D + 1)], F32R, name=f"vaug{t}")
                      for t in range(NTT)]
            for t in range(NTT):
                for h in range(NHPC):
                    col = h * (D + 1) + D
                    nc.vector.tensor_copy(vaug_t[t][:, col : col + 1], ones_f32[:])
            # normalized O^T [dims, tokens]
            ot_t = [persist.tile([P, T], F32R, name=f"ot{p}") for p in range(2)]

            def emit_weight_dmas():
                for c in range(NCT):
                    nc.sync.dma_start(wq_t[c][:], wqkv[c * P : (c + 1) * P, :])
                for k in range(2):
                    nc.sync.dma_start(wot_t[k][:], wot[k * P : (k + 1) * P, :])

            def emit_stage1():
                with (
                    tc.tile_pool(name="xt", bufs=24) as xt_pool,
                    tc.tile_pool(name="psproj", bufs=4, space="PSUM") as ps_proj,
                ):
                    for n in range(NCHUNK):
                        csl = slice(n * CH, (n + 1) * CH)
                        xts = []
                        for c in range(NCT):
                            xtile = xt_pool.tile([P, CH], F32R, tag="xt",
                                                 name=f"xt{n}_{c}")
                            nc.sync.dma_start(xtile[:], xT[c * P : (c + 1) * P, csl])
                            xts.append(xtile)
                        if n == 0:
                            # weights go to the DMA queues after chunk-0 x tiles
                            emit_weight_dmas()
                        # Q^T/K^T: W stationary, x^T moving -> [dims, tokens]
                        for m in range(4):
                            ps = ps_proj.tile([P, CH], F32, tag="ps",
                                              name=f"ps{n}_{m}")
                            for c in range(NCT):
                                nc.tensor.matmul(
                                    ps[:],
                                    lhsT=wq_t[c][:, m * P : (m + 1) * P],
                                    rhs=xts[c][:],
                                    start=(c == 0),
                                    stop=(c == NCT - 1),
                                )
                            if skip_proj_copies:
                                continue
                            if m < 2:
                                nc.vector.tensor_copy(qt_t[m][:, csl], ps[:])
                            else:
                                nc.vector.tensor_copy(kt_t[m - 2][:, csl], ps[:])
                        if skip_v:
                            continue
                        # V natural: x^T tile stationary, Wv moving -> [tokens, vdims]
                        for j in range(4):
                            vp = ps_proj.tile([P, DH], F32, tag="vp",
                                              name=f"vp{n}_{j}")
                            for c in range(NCT):
                                nc.tensor.matmul(
                                    vp[:],
                                    lhsT=xts[c][:, j * P : (j + 1) * P],
                                    rhs=wq_t[c][:, 2 * DH : 3 * DH],
                                    start=(c == 0),
                                    stop=(c == NCT - 1),
                                )
                            if skip_proj_copies:
                                continue
                            # scatter vdims into the per-head 65-col layout
                            va = vaug_t[4 * n + j]
                            for h in range(NHPC):
                                nc.vector.tensor_copy(
                                    va[:, h * (D + 1) : h * (D + 1) + D],
                                    vp[:, h * D : (h + 1) * D])

            def emit_stage23():
                with (
                    tc.tile_pool(name="pt", bufs=4) as pt_pool,
                    tc.tile_pool(name="small", bufs=4) as small_pool,
                    tc.tile_pool(name="ysb", bufs=3) as y_pool,
                    tc.tile_pool(name="psst", bufs=2, space="PSUM") as ps_st,
                    tc.tile_pool(name="psot", bufs=2, space="PSUM") as ps_ot,
                    tc.tile_pool(name="psy", bufs=2, space="PSUM") as ps_y,
                ):
                    pending_out = []

                    def emit_outproj_tile(tt):
                        tsl = slice(tt * P, (tt + 1) * P)
                        for nn in range(2):
                            nsl = slice(nn * CH, (nn + 1) * CH)
                            yp = ps_y.tile([P, CH], F32, tag="y",
                                           name=f"y{tt}_{nn}")
                            for k in range(2):
                                nc.tensor.matmul(
                                    yp[:],
                                    lhsT=ot_t[k][:, tsl],
                                    rhs=wot_t[k][:, nsl],
                                    start=(k == 0),
                                    stop=(k == 1),
                                )
                            ysb = y_pool.tile([P, CH], F32, tag="ysb",
                                              name=f"ysb{tt}_{nn}")
                            if (tt + nn) % 2 == 0:
                                nc.vector.tensor_copy(ysb[:], yp[:])
                            else:
                                nc.scalar.copy(ysb[:], yp[:])
                            nc.sync.dma_start(y[tsl, nsl], ysb[:])

                    for cq in range(NCHUNK):
                        qsl = slice(cq * CH, (cq + 1) * CH)
                        nts = 4 * cq + 4

                        def emit_st(t, cq=cq, p=None):
                            st = ps_st.tile([P, 2, CH], F32, tag="st",
                                            name=f"st{cq}_{p}_{t}")
                            tsl = slice(t * P, (t + 1) * P)
                            js = max(0, (t - 4 * cq) * P)
                            qs = slice(cq * CH + js, (cq + 1) * CH)
                            for hh in range(2):
                                nc.tensor.matmul(
                                    st[:, hh, js:],
                                    lhsT=kt_t[p][hh * D : (hh + 1) * D, tsl],
                                    rhs=qt_t[p][hh * D : (hh + 1) * D, qs],
                                    start=True,
                                    stop=True,
                                )
                            if t >= 4 * cq and not no_mask:
                                nc.vector.tensor_add(
                                    st[:, :, js : js + P],
                                    st[:, :, js : js + P],
                                    mask128[:],
                                )
                            return st, js

                        for p in range(2):
                            ots = [
                                ps_ot.tile([D + 1, CH], F32, tag="ot",
                                           name=f"ot{cq}_{p}_{hh}")
                                for hh in range(2)
                            ]
                            sts = {0: emit_st(0, p=p)}
                            for t in range(nts):
                                # pipeline: next tile's scores go ahead of AV
                                if t + 1 < nts:
                                    sts[t + 1] = emit_st(t + 1, p=p)
                                st, js = sts.pop(t)
                                pt = pt_pool.tile([P, 2, CH], F32R, tag="pt",
                                                  name=f"pt{cq}_{p}_{t}")
                                nc.scalar.activation(pt[:, :, js:], st[:, :, js:],
                                                     EXP)
                                for hh in range(2):
                                    h = 2 * p + hh
                                    nc.tensor.matmul(
                                        ots[hh][:, js:],
                                        lhsT=vaug_t[t][:, h * (D + 1)
                                                       : (h + 1) * (D + 1)],
                                        rhs=pt[:, hh, js:],
                                        start=(t == 0),
                                        stop=(t == nts - 1),
                                    )
                                # spread previous chunk's out-projection
                                if pending_out and t >= 1:
                                    emit_outproj_tile(pending_out.pop(0))
                            for hh in range(2):
                                ot = ots[hh]
                                if no_norm:
                                    with nc.allow_low_precision("timing variant"):
                                        nc.vector.tensor_copy(
                                            ot_t[p][hh * D : (hh + 1) * D, qsl],
                                            ot[0:D, :])
                                    continue
                                # bounce to SBUF (frees the PSUM slot quickly)
                                otu = small_pool.tile([D + 1, CH], F32, tag="otu",
                                                      name=f"otu{cq}_{p}_{hh}")
                                nc.vector.tensor_copy(otu[:], ot[:])
                                # 1/denom at partition 0, then GPSIMD broadcast
                                recip = small_pool.tile([1, CH], F32, tag="rc",
                                                        name=f"rc{cq}_{p}_{hh}")
                                nc.vector.reciprocal(recip[:], otu[D : D + 1, :])
                                den = small_pool.tile([D, CH], F32, tag="den",
                                                      name=f"dn{cq}_{p}_{hh}")
                                nc.gpsimd.partition_broadcast(den[:], recip[:])
                                with nc.allow_low_precision("fp32r store"):
                                    nc.vector.tensor_mul(
                                        ot_t[p][hh * D : (hh + 1) * D, qsl],
                                        otu[0:D, :],
                                        den[:],
                                    )
                        if stages >= 3:
                            pending_out.extend(range(4 * cq, 4 * cq + 4))
                    while pending_out:
                        emit_outproj_tile(pending_out.pop(0))

            def emit_dbg_outputs():
                if stages == 1:
                    dbg_srcs = ((wq_t[0], wq_t[1], wq_t[2], wq_t[3])
                                if skip_proj_copies else
                                (qt_t[0], qt_t[1], kt_t[0], kt_t[1]))
                    for i, src_t in enumerate(dbg_srcs):
                        w = min(C, src_t.shape[1])
                        nc.sync.dma_start(y[i * P : (i + 1) * P, 0:w],
                                          src_t[:, 0:w].bitcast(F32))
                    with tc.tile_pool(name="dbg", bufs=2) as dbgp:
                        for tt in range(4):
                            db = dbgp.tile([P, C], F32, tag="db", name=f"db{tt}")
                            nc.gpsimd.memset(db[:], 0.0)
                            for j in range(4):
                                nc.vector.tensor_copy(
                                    db[:, j * 256 : j * 256 + 256],
                                    vaug_t[4 * tt + j][:, 0:256].bitcast(F32))
                            nc.sync.dma_start(y[(4 + tt) * P : (5 + tt) * P, :],
                                              db[:])
                elif stages == 2:
                    for i, src_t in enumerate((ot_t[0], ot_t[1])):
                        nc.sync.dma_start(y[i * P : (i + 1) * P, :],
                                          src_t[:, 0:C].bitcast(F32))

            def emit_body():
                emit_stage1()
                if stages >= 2:
                    emit_stage23()
                emit_dbg_outputs()

            if loop_reps is None:
                emit_body()
            else:
                with tc.For_i(0, loop_reps, 1):
                    emit_body()

    nc.finalize()
    return nc


_NC_CACHE = None


def get_nc():
    global _NC_CACHE
    if _NC_CACHE is None:
        _NC_CACHE = build_nc()
    return _NC_CACHE


def make_in_maps(x, Wq, Wk, Wv, Wo):
    scale = 1.0 / np.sqrt(np.float32(C))
    in_maps = []
    for core in range(8):
        b, hg = core // 4, core % 4
        hsl = slice(hg * NHPC, (hg + 1) * NHPC)
        xT = np.ascontiguousarray(x[b].T)
        wq = (Wq[hsl] * scale).transpose(1, 0, 2).reshape(C, DH)
        wk = Wk[hsl].transpose(1, 0, 2).reshape(C, DH)
        wv = Wv[hsl].transpose(1, 0, 2).reshape(C, DH)
        wqkv = np.ascontiguousarray(
            np.concatenate([wq, wk, wv], axis=1, dtype=np.float32))
        wot = np.ascontiguousarray(Wo[:, hg * DH : (hg + 1) * DH].T)
        in_maps.append({
            "xT": xT.astype(np.float32, copy=False),
            "Wqkv": wqkv,
            "WoT": wot.astype(np.float32, copy=False),
        })
    return in_maps


def gather(results, bo):
    out = np.zeros((B, T, C), dtype=np.float32)
    for core in range(8):
        out[core // 4] += results[core]["Y"]
    out += bo.astype(np.float32)
    return out


def kernel(x, Wq, Wk, Wv, Wo, bo, **run_kwargs):
    x = np.asarray(x, dtype=np.float32)
    Wq = np.asarray(Wq, dtype=np.float32)
    Wk = np.asarray(Wk, dtype=np.float32)
    Wv = np.asarray(Wv, dtype=np.float32)
    Wo = np.asarray(Wo, dtype=np.float32)
    bo = np.asarray(bo, dtype=np.float32)
    nc = get_nc()
    in_maps = make_in_maps(x, Wq, Wk, Wv, Wo)
    res = run_bass_kernel_spmd(nc, in_maps, core_ids=list(range(8)), **run_kwargs)
    out = gather(res.results, bo)
    if run_kwargs:
        return out, res
    return out



# revision 37
# speedup vs baseline: 24105.4893x; 24105.4893x over previous
"""Trainium2 Bass kernel for 16-head causal MHA (B=2, T=2048, C=1024, H=16, D=64).

Sharding: 8 cores = 2 batch groups x 4 head groups (4 heads each).
Each core computes, for its batch b and heads hg*4..hg*4+3:
  Q^T,K^T = projections kept transposed [dims, tokens] (fp32r matmuls)
  V       = projection transposed back to [tokens, dims] via PE transpose,
            augmented with a ones column per head (denominator trick)
  S^T     = K Q^T per (ts-tile, tq-chunk), causal-masked on the diagonal
            128-block only (fully-masked columns skipped), exp'd (scale
            folded into Wq on host)
  O^T_aug = V_aug^T P^T accumulated over ts tiles; row 64 is the softmax
            denominator; normalized via GPSIMD partition_broadcast + DVE
  Y_part  = O^T.T @ Wo_slice^T, interleaved per chunk  [2048, 1024]
Host sums the 4 head-group partials per batch and adds bo.
"""

import sys

sys.path.insert(0, "/opt/trn_rl_repo")

import numpy as np

import concourse.bass as bass
from concourse import bacc
import concourse.mybir as mybir
from concourse.tile import TileContext
from concourse.bass_utils import run_bass_kernel_spmd
from concourse.masks import make_identity

F32 = mybir.dt.float32
F32R = mybir.dt.float32r
BF16 = mybir.dt.bfloat16
EXP = mybir.ActivationFunctionType.Exp

B, T, C, H, D = 2, 2048, 1024, 16, 64
NHPC = 4          # heads per core
DH = NHPC * D     # 256 head dims per core
P = 128           # partitions
CH = 512          # token chunk (matmul moving dim)
NCHUNK = T // CH  # 4
NTT = T // P      # 16 token tiles
NCT = C // P      # 8 contraction tiles over C
NEG = -1.0e10


def build_nc(loop_reps=None, stages=3, no_mask=False, no_norm=False,
             interleave_out=True, skip_v=False, skip_proj_copies=False):
    nc = bacc.Bacc()
    xT_d = nc.declare_dram_parameter("xT", [C, T], BF16, isOutput=False)
    wqkv_d = nc.declare_dram_parameter("Wqkv", [C, 3 * DH], BF16, isOutput=False)
    wot_d = nc.declare_dram_parameter("WoT", [DH, C], BF16, isOutput=False)
    y_d = nc.declare_dram_parameter("Y", [T, C], F32, isOutput=True)

    xT = xT_d[:, :]
    wqkv = wqkv_d[:, :]
    wot = wot_d[:, :]
    y = y_d[:, :]

    with TileContext(nc) as tc:
        with (
            tc.tile_pool(name="const", bufs=1) as const,
            tc.tile_pool(name="persist", bufs=1) as persist,
        ):
            # ---- constants ----
            ones_f32 = const.tile([P, 1], F32)
            nc.gpsimd.memset(ones_f32[:], 1.0)
            ones_row = const.tile([1, D], F32)
            nc.gpsimd.memset(ones_row[:], 1.0)
            ones_col = const.tile([1, D], F32R)
            nc.vector.tensor_copy(ones_col[:], ones_row[:])
            # triangular mask for the diagonal 128x128 block (both halves):
            # mask[r, (half, j)] = 0 if r <= j else -1e10
            # warm the GPSIMD library that partition_broadcast lives in —
            # otherwise the first broadcast triggers a ~7us LOAD_LIB swap
            # in the middle of stage 2
            wsrc = const.tile([1, 8], F32, name="warmsrc")
            wdst = const.tile([D, 8], F32, name="warmdst")
            nc.gpsimd.memset(wsrc[:], 1.0)
            nc.gpsimd.partition_broadcast(wdst[:], wsrc[:])
            nc.vector.tensor_copy(wsrc[:], wdst[0:1, :])
            mask128 = const.tile([P, 2, P], F32, name="mask128")
            nc.gpsimd.memset(mask128[:], 0.0)
            nc.gpsimd.affine_select(
                out=mask128[:],
                in_=mask128[:],
                compare_op=mybir.AluOpType.is_ge,
                fill=NEG,
                base=0,
                pattern=[[0, 2], [1, P]],
                channel_multiplier=-1,
            )

            # ---- persistent tensors ----
            wq_t = [persist.tile([P, 3 * DH], BF16, name=f"wqkv{c}")
                    for c in range(NCT)]
            # stage-2/3 pipeline runs in bf16: halves score/out-proj
            # LDWEIGHTS via FWL and avoids the 4x fp32r penalty on the
            # 128-wide diagonal tiles; accumulation stays fp32 in PSUM
            wot_t = [persist.tile([P, C], BF16, name=f"wot{k}") for k in range(2)]
            # Q^T/K^T [dims, tokens]; pair p holds heads (2p, 2p+1)
            qt_t = [persist.tile([P, T], BF16, name=f"qt{p}") for p in range(2)]
            kt_t = [persist.tile([P, T], BF16, name=f"kt{p}") for p in range(2)]
            # V augmented with a ones column per head: [tokens, 4*65]
            vaug_t = [persist.tile([P, NHPC * (D + 1)], BF16, name=f"vaug{t}")
                      for t in range(NTT)]
            with nc.allow_low_precision("bf16 attention pipeline"):
                for t in range(NTT):
                    for h in range(NHPC):
                        col = h * (D + 1) + D
                        nc.vector.tensor_copy(vaug_t[t][:, col : col + 1],
                                              ones_f32[:])
            # normalized O^T [dims, tokens]
            ot_t = [persist.tile([P, T], BF16, name=f"ot{p}") for p in range(2)]

            def emit_stage1():
                with (
                    tc.tile_pool(name="xt", bufs=24) as xt_pool,
                    tc.tile_pool(name="psproj", bufs=4, space="PSUM") as ps_proj,
                ):
                    for n in range(NCHUNK):
                        csl = slice(n * CH, (n + 1) * CH)
                        xts = []
                        for c in range(NCT):
                            if n == 0:
                                # Q/K weight cols on the sync queue first (the
                                # V cols follow) while x tiles stream on the
                                # scalar queue: both first matmul operands land
                                # early and in parallel
                                nc.sync.dma_start(
                                    wq_t[c][:, 0 : 2 * DH],
                                    wqkv[c * P : (c + 1) * P, 0 : 2 * DH])
                            xtile = xt_pool.tile([P, CH], BF16, tag="xt",
                                                 name=f"xt{n}_{c}")
                            nc.scalar.dma_start(xtile[:],
                                                xT[c * P : (c + 1) * P, csl])
                            xts.append(xtile)
                        if n == 0:
                            for c in range(NCT):
                                nc.sync.dma_start(
                                    wq_t[c][:, 2 * DH : 3 * DH],
                                    wqkv[c * P : (c + 1) * P, 2 * DH : 3 * DH])
                            # out-proj weights are needed only by stage 3
                            for k in range(2):
                                nc.sync.dma_start(wot_t[k][:],
                                                  wot[k * P : (k + 1) * P, :])
                        # Q^T/K^T: W stationary, x^T moving -> [dims, tokens]
                        for m in range(4):
                            ps = ps_proj.tile([P, CH], F32, tag="ps",
                                              name=f"ps{n}_{m}")
                            for c in range(NCT):
                                nc.tensor.matmul(
                                    ps[:],
                                    lhsT=wq_t[c][:, m * P : (m + 1) * P],
                                    rhs=xts[c][:],
                                    start=(c == 0),
                                    stop=(c == NCT - 1),
                                )
                            if skip_proj_copies:
                                continue
                            with nc.allow_low_precision("bf16 Q/K"):
                                if m < 2:
                                    nc.vector.tensor_copy(qt_t[m][:, csl], ps[:])
                                else:
                                    nc.vector.tensor_copy(kt_t[m - 2][:, csl],
                                                          ps[:])
                        if skip_v:
                            continue
                        # V natural: x^T tile stationary, Wv moving -> [tokens, vdims]
                        for j in range(4):
                            vp = ps_proj.tile([P, DH], F32, tag="vp",
                                              name=f"vp{n}_{j}")
                            for c in range(NCT):
                                nc.tensor.matmul(
                                    vp[:],
                                    lhsT=xts[c][:, j * P : (j + 1) * P],
                                    rhs=wq_t[c][:, 2 * DH : 3 * DH],
                                    start=(c == 0),
                                    stop=(c == NCT - 1),
                                )
                            if skip_proj_copies:
                                continue
                            # scatter vdims into the per-head 65-col layout
                            va = vaug_t[4 * n + j]
                            with nc.allow_low_precision("bf16 V"):
                                for h in range(NHPC):
                                    nc.vector.tensor_copy(
                                        va[:, h * (D + 1) : h * (D + 1) + D],
                                        vp[:, h * D : (h + 1) * D])

            def emit_stage23():
                with (
                    tc.tile_pool(name="pt", bufs=4) as pt_pool,
                    tc.tile_pool(name="small", bufs=4) as small_pool,
                    tc.tile_pool(name="ysb", bufs=3) as y_pool,
                    tc.tile_pool(name="psst", bufs=2, space="PSUM") as ps_st,
                    # 3 ot buffers: the next head-pair's AV accumulation can
                    # start while the previous pair's normalize still holds one
                    tc.tile_pool(name="psot", bufs=3, space="PSUM") as ps_ot,
                    tc.tile_pool(name="psy", bufs=1, space="PSUM") as ps_y,
                ):
                    pending_out = []

                    def emit_outproj_tile(tt, on_scalar=False, split_k1=False):
                        tsl = slice(tt * P, (tt + 1) * P)
                        for nn in range(2):
                            nsl = slice(nn * CH, (nn + 1) * CH)
                            yp = ps_y.tile([P, CH], F32, tag="y",
                                           name=f"y{tt}_{nn}")
                            nc.tensor.matmul(
                                yp[:],
                                lhsT=ot_t[0][:, tsl],
                                rhs=wot_t[0][:, nsl],
                                start=True,
                                stop=False,
                            )
                            if split_k1:
                                # two K=64 halves: each head-pair half can
                                # start as soon as its normalize lands
                                for hh in range(2):
                                    nc.tensor.matmul(
                                        yp[:],
                                        lhsT=ot_t[1][hh * D : (hh + 1) * D, tsl],
                                        rhs=wot_t[1][hh * D : (hh + 1) * D, nsl],
                                        start=False,
                                        stop=(hh == 1),
                                    )
                            else:
                                nc.tensor.matmul(
                                    yp[:],
                                    lhsT=ot_t[1][:, tsl],
                                    rhs=wot_t[1][:, nsl],
                                    start=False,
                                    stop=True,
                                )
                            ysb = y_pool.tile([P, CH], F32, tag="ysb",
                                              name=f"ysb{tt}_{nn}")
                            # mid-kernel Y copies stay off the Scalar engine
                            # (exp keeps ACT in lockstep with the PE); the
                            # final drain uses the then-idle Scalar engine
                            if on_scalar:
                                nc.scalar.copy(ysb[:], yp[:])
                            else:
                                nc.vector.tensor_copy(ysb[:], yp[:])
                            nc.sync.dma_start(y[tsl, nsl], ysb[:])

                    def emit_st(cq, p, t):
                        st = ps_st.tile([P, 2, CH], F32, tag="st",
                                        name=f"st{cq}_{p}_{t}")
                        tsl = slice(t * P, (t + 1) * P)
                        js = max(0, (t - 4 * cq) * P)
                        qs = slice(cq * CH + js, (cq + 1) * CH)
                        for hh in range(2):
                            nc.tensor.matmul(
                                st[:, hh, js:],
                                lhsT=kt_t[p][hh * D : (hh + 1) * D, tsl],
                                rhs=qt_t[p][hh * D : (hh + 1) * D, qs],
                                start=True,
                                stop=True,
                            )
                        if t >= 4 * cq and not no_mask:
                            nc.vector.tensor_add(
                                st[:, :, js : js + P],
                                st[:, :, js : js + P],
                                mask128[:],
                            )
                        return st, js

                    def norm_steps(cq, p, ots, on_scalar=False):
                        """Yield the normalize chain for (cq, p) as 5 separate
                        steps, so callers can interleave them with the next
                        group's mask adds on the DVE queue."""
                        qsl = slice(cq * CH, (cq + 1) * CH)
                        if no_norm:
                            def all_copies():
                                with nc.allow_low_precision("timing variant"):
                                    for hh in range(2):
                                        nc.vector.tensor_copy(
                                            ot_t[p][hh * D : (hh + 1) * D, qsl],
                                            ots[hh][0:D, :])
                            return [all_copies]
                        cp = nc.scalar.copy if on_scalar else nc.vector.tensor_copy
                        state = {}

                        def phase1(hh):
                            def fn():
                                ot = ots[hh]
                                # denom row straight from PSUM to partition 0
                                # (the custom-DVE approx op needs a partition-0
                                # source); runs parallel to the otu bounce
                                dstg = small_pool.tile([1, CH], F32, tag="ds",
                                                       name=f"ds{cq}_{p}_{hh}")
                                cp(dstg[:], ot[D : D + 1, :])
                                # bounce to SBUF (frees the PSUM slot quickly)
                                otu = small_pool.tile([D, CH], F32, tag="otu",
                                                      name=f"otu{cq}_{p}_{hh}")
                                cp(otu[:], ot[0:D, :])
                                state[hh] = (dstg, otu)
                            return fn

                        def recips():
                            for hh in range(2):
                                dstg, otu = state[hh]
                                recip = small_pool.tile([1, CH], F32, tag="rc",
                                                        name=f"rc{cq}_{p}_{hh}")
                                nc.vector.reciprocal_approx_fast(
                                    recip[:], dstg[:])
                                state[hh] = (recip, otu)

                        def phase2(hh):
                            def fn():
                                recip, otu = state[hh]
                                den = small_pool.tile([D, CH], F32, tag="den",
                                                      name=f"dn{cq}_{p}_{hh}")
                                nc.gpsimd.partition_broadcast(den[:], recip[:])
                                with nc.allow_low_precision("fp32r store"):
                                    nc.vector.tensor_mul(
                                        ot_t[p][hh * D : (hh + 1) * D, qsl],
                                        otu[:],
                                        den[:],
                                    )
                            return fn

                        return [phase1(0), phase1(1), recips,
                                phase2(0), phase2(1)]

                    # one flat score/exp/AV pipeline across every (chunk,
                    # head-pair) boundary — the score matmuls for the next
                    # group fill the PE while the previous group's exp and
                    # normalize complete
                    tiles = [(cq, p, t)
                             for cq in range(NCHUNK)
                             for p in range(2)
                             for t in range(4 * cq + 4)]
                    ots_cur = None
                    norm_q = []     # deferred normalize steps, one per flat step
                    sts = {tiles[0]: emit_st(*tiles[0])}
                    for i, (cq, p, t) in enumerate(tiles):
                        nts = 4 * cq + 4
                        if t == 0:
                            ots_cur = [
                                ps_ot.tile([D + 1, CH], F32, tag="ot",
                                           name=f"ot{cq}_{p}_{hh}")
                                for hh in range(2)
                            ]
                        if i + 1 < len(tiles):
                            sts[tiles[i + 1]] = emit_st(*tiles[i + 1])
                        # one deferred normalize step per flat step: the DVE
                        # queue interleaves them with the diag mask adds
                        # instead of stalling exp behind the whole chain
                        if norm_q:
                            norm_q.pop(0)()
                        st, js = sts.pop((cq, p, t))
                        pt = pt_pool.tile([P, 2, CH], BF16, tag="pt",
                                          name=f"pt{cq}_{p}_{t}")
                        with nc.allow_low_precision("bf16 probs"):
                            nc.scalar.activation(pt[:, :, js:],
                                                 st[:, :, js:], EXP)
                        for hh in range(2):
                            h = 2 * p + hh
                            nc.tensor.matmul(
                                ots_cur[hh][:, js:],
                                lhsT=vaug_t[t][:, h * (D + 1)
                                               : (h + 1) * (D + 1)],
                                rhs=pt[:, hh, js:],
                                start=(t == 0),
                                stop=(t == nts - 1),
                            )
                        # spread previous chunk's out-projection; start late
                        # so the normalize chain never blocks the PE
                        if pending_out and t >= 6:
                            emit_outproj_tile(pending_out.pop(0))
                        if t == nts - 1:
                            norm_q.extend(norm_steps(
                                cq, p, ots_cur,
                                on_scalar=(i == len(tiles) - 1)))
                            if p == 1 and stages >= 3:
                                pending_out.extend(range(4 * cq, 4 * cq + 4))
                    # the last group's normalize runs with no exp work left:
                    # emit the remaining steps immediately on the Scalar engine
                    for fn in norm_q:
                        fn()
                    while pending_out:
                        emit_outproj_tile(pending_out.pop(0), on_scalar=True)

            def emit_dbg_outputs():
                if stages == 1:
                    dbg_srcs = ((wq_t[0], wq_t[1], wq_t[2], wq_t[3])
                                if skip_proj_copies else
                                (qt_t[0], qt_t[1], kt_t[0], kt_t[1]))
                    for i, src_t in enumerate(dbg_srcs):
                        w = min(C, src_t.shape[1])
                        nc.sync.dma_start(y[i * P : (i + 1) * P, 0:w],
                                          src_t[:, 0:w].bitcast(F32))
                    with tc.tile_pool(name="dbg", bufs=2) as dbgp:
                        for tt in range(4):
                            db = dbgp.tile([P, C], F32, tag="db", name=f"db{tt}")
                            nc.gpsimd.memset(db[:], 0.0)
                            for j in range(4):
                                nc.vector.tensor_copy(
                                    db[:, j * 256 : j * 256 + 256],
                                    vaug_t[4 * tt + j][:, 0:256].bitcast(F32))
                            nc.sync.dma_start(y[(4 + tt) * P : (5 + tt) * P, :],
                                              db[:])
                elif stages == 2:
                    for i, src_t in enumerate((ot_t[0], ot_t[1])):
                        nc.sync.dma_start(y[i * P : (i + 1) * P, :],
                                          src_t[:, 0:C].bitcast(F32))

            def emit_body():
                emit_stage1()
                if stages >= 2:
                    emit_stage23()
                emit_dbg_outputs()

            if loop_reps is None:
                emit_body()
            else:
                with tc.For_i(0, loop_reps, 1):
                    emit_body()

    nc.finalize()
    return nc


_NC_CACHE = None


def get_nc():
    global _NC_CACHE
    if _NC_CACHE is None:
        _NC_CACHE = build_nc()
    return _NC_CACHE


def make_in_maps(x, Wq, Wk, Wv, Wo):
    import ml_dtypes

    bf16 = ml_dtypes.bfloat16
    scale = 1.0 / np.sqrt(np.float32(C))
    in_maps = []
    for core in range(8):
        b, hg = core // 4, core % 4
        hsl = slice(hg * NHPC, (hg + 1) * NHPC)
        xT = np.ascontiguousarray(x[b].T).astype(bf16)
        wq = (Wq[hsl] * scale).transpose(1, 0, 2).reshape(C, DH)
        wk = Wk[hsl].transpose(1, 0, 2).reshape(C, DH)
        wv = Wv[hsl].transpose(1, 0, 2).reshape(C, DH)
        wqkv = np.ascontiguousarray(
            np.concatenate([wq, wk, wv], axis=1, dtype=np.float32)).astype(bf16)
        wot = np.ascontiguousarray(Wo[:, hg * DH : (hg + 1) * DH].T).astype(bf16)
        in_maps.append({
            "xT": xT,
            "Wqkv": wqkv,
            "WoT": wot,
        })
    return in_maps


def gather(results, bo):
    out = np.zeros((B, T, C), dtype=np.float32)
    for core in range(8):
        out[core // 4] += results[core]["Y"]
    out += bo.astype(np.float32)
    return out


def kernel(x, Wq, Wk, Wv, Wo, bo, **run_kwargs):
    x = np.asarray(x, dtype=np.float32)
    Wq = np.asarray(Wq, dtype=np.float32)
    Wk = np.asarray(Wk, dtype=np.float32)
    Wv = np.asarray(Wv, dtype=np.float32)
    Wo = np.asarray(Wo, dtype=np.float32)
    bo = np.asarray(bo, dtype=np.float32)
    nc = get_nc()
    in_maps = make_in_maps(x, Wq, Wk, Wv, Wo)
    res = run_bass_kernel_spmd(nc, in_maps, core_ids=list(range(8)), **run_kwargs)
    out = gather(res.results, bo)
    if run_kwargs:
        return out, res
    return out



# revision 39
# speedup vs baseline: 24525.9850x; 1.0174x over previous
"""Trainium2 Bass kernel for 16-head causal MHA (B=2, T=2048, C=1024, H=16, D=64).

Sharding: 8 cores = 2 batch groups x 4 head groups (4 heads each).
Each core computes, for its batch b and heads hg*4..hg*4+3:
  Q^T,K^T = projections kept transposed [dims, tokens] (fp32r matmuls)
  V       = projection transposed back to [tokens, dims] via PE transpose,
            augmented with a ones column per head (denominator trick)
  S^T     = K Q^T per (ts-tile, tq-chunk), causal-masked on the diagonal
            128-block only (fully-masked columns skipped), exp'd (scale
            folded into Wq on host)
  O^T_aug = V_aug^T P^T accumulated over ts tiles; row 64 is the softmax
            denominator; normalized via GPSIMD partition_broadcast + DVE
  Y_part  = O^T.T @ Wo_slice^T, interleaved per chunk  [2048, 1024]
Host sums the 4 head-group partials per batch and adds bo.
"""

import sys

sys.path.insert(0, "/opt/trn_rl_repo")

import numpy as np

import concourse.bass as bass
from concourse import bacc
import concourse.mybir as mybir
from concourse.tile import TileContext
from concourse.bass_utils import run_bass_kernel_spmd
from concourse.masks import make_identity

F32 = mybir.dt.float32
F32R = mybir.dt.float32r
BF16 = mybir.dt.bfloat16
EXP = mybir.ActivationFunctionType.Exp

B, T, C, H, D = 2, 2048, 1024, 16, 64
NHPC = 4          # heads per core
DH = NHPC * D     # 256 head dims per core
P = 128           # partitions
CH = 512          # token chunk (matmul moving dim)
NCHUNK = T // CH  # 4
NTT = T // P      # 16 token tiles
NCT = C // P      # 8 contraction tiles over C
NEG = -1.0e10


def build_nc(loop_reps=None, stages=3, no_mask=False, no_norm=False,
             interleave_out=True, skip_v=False, skip_proj_copies=False):
    nc = bacc.Bacc()
    xT_d = nc.declare_dram_parameter("xT", [C, T], BF16, isOutput=False)
    wqkv_d = nc.declare_dram_parameter("Wqkv", [C, 3 * DH], BF16, isOutput=False)
    wot_d = nc.declare_dram_parameter("WoT", [DH, C], BF16, isOutput=False)
    y_d = nc.declare_dram_parameter("Y", [T, C], F32, isOutput=True)

    xT = xT_d[:, :]
    wqkv = wqkv_d[:, :]
    wot = wot_d[:, :]
    y = y_d[:, :]

    with TileContext(nc) as tc:
        with (
            tc.tile_pool(name="const", bufs=1) as const,
            tc.tile_pool(name="persist", bufs=1) as persist,
        ):
            # ---- constants ----
            ones_f32 = const.tile([P, 1], F32)
            nc.gpsimd.memset(ones_f32[:], 1.0)
            ones_row = const.tile([1, D], F32)
            nc.gpsimd.memset(ones_row[:], 1.0)
            ones_col = const.tile([1, D], F32R)
            nc.vector.tensor_copy(ones_col[:], ones_row[:])
            # triangular mask for the diagonal 128x128 block (both halves):
            # mask[r, (half, j)] = 0 if r <= j else -1e10
            # warm the GPSIMD library that partition_broadcast lives in —
            # otherwise the first broadcast triggers a ~7us LOAD_LIB swap
            # in the middle of stage 2
            wsrc = const.tile([1, 8], F32, name="warmsrc")
            wdst = const.tile([D, 8], F32, name="warmdst")
            nc.gpsimd.memset(wsrc[:], 1.0)
            nc.gpsimd.partition_broadcast(wdst[:], wsrc[:])
            nc.vector.tensor_copy(wsrc[:], wdst[0:1, :])
            mask128 = const.tile([P, 2, P], F32, name="mask128")
            nc.gpsimd.memset(mask128[:], 0.0)
            nc.gpsimd.affine_select(
                out=mask128[:],
                in_=mask128[:],
                compare_op=mybir.AluOpType.is_ge,
                fill=NEG,
                base=0,
                pattern=[[0, 2], [1, P]],
                channel_multiplier=-1,
            )

            # ---- persistent tensors ----
            wq_t = [persist.tile([P, 3 * DH], BF16, name=f"wqkv{c}")
                    for c in range(NCT)]
            # stage-2/3 pipeline runs in bf16: halves score/out-proj
            # LDWEIGHTS via FWL and avoids the 4x fp32r penalty on the
            # 128-wide diagonal tiles; accumulation stays fp32 in PSUM
            wot_t = [persist.tile([P, C], BF16, name=f"wot{k}") for k in range(2)]
            # Q^T/K^T [dims, tokens]; pair p holds heads (2p, 2p+1)
            qt_t = [persist.tile([P, T], BF16, name=f"qt{p}") for p in range(2)]
            kt_t = [persist.tile([P, T], BF16, name=f"kt{p}") for p in range(2)]
            # V augmented with a ones column per head: [tokens, 4*65]
            vaug_t = [persist.tile([P, NHPC * (D + 1)], BF16, name=f"vaug{t}")
                      for t in range(NTT)]
            with nc.allow_low_precision("bf16 attention pipeline"):
                for t in range(NTT):
                    for h in range(NHPC):
                        col = h * (D + 1) + D
                        nc.vector.tensor_copy(vaug_t[t][:, col : col + 1],
                                              ones_f32[:])
            # normalized O^T [dims, tokens]
            ot_t = [persist.tile([P, T], BF16, name=f"ot{p}") for p in range(2)]

            def emit_stage1():
                with (
                    tc.tile_pool(name="xt", bufs=24) as xt_pool,
                    tc.tile_pool(name="psproj", bufs=4, space="PSUM") as ps_proj,
                ):
                    for n in range(NCHUNK):
                        csl = slice(n * CH, (n + 1) * CH)
                        xts = []
                        for c in range(NCT):
                            if n == 0:
                                # Q/K weight cols on the sync queue first (the
                                # V cols follow) while x tiles stream on the
                                # scalar queue: both first matmul operands land
                                # early and in parallel
                                nc.sync.dma_start(
                                    wq_t[c][:, 0 : 2 * DH],
                                    wqkv[c * P : (c + 1) * P, 0 : 2 * DH])
                            xtile = xt_pool.tile([P, CH], BF16, tag="xt",
                                                 name=f"xt{n}_{c}")
                            nc.scalar.dma_start(xtile[:],
                                                xT[c * P : (c + 1) * P, csl])
                            xts.append(xtile)
                        if n == 0:
                            for c in range(NCT):
                                nc.sync.dma_start(
                                    wq_t[c][:, 2 * DH : 3 * DH],
                                    wqkv[c * P : (c + 1) * P, 2 * DH : 3 * DH])
                            # out-proj weights are needed only by stage 3
                            for k in range(2):
                                nc.sync.dma_start(wot_t[k][:],
                                                  wot[k * P : (k + 1) * P, :])
                        # Q^T/K^T: W stationary, x^T moving -> [dims, tokens]
                        for m in range(4):
                            ps = ps_proj.tile([P, CH], F32, tag="ps",
                                              name=f"ps{n}_{m}")
                            for c in range(NCT):
                                nc.tensor.matmul(
                                    ps[:],
                                    lhsT=wq_t[c][:, m * P : (m + 1) * P],
                                    rhs=xts[c][:],
                                    start=(c == 0),
                                    stop=(c == NCT - 1),
                                )
                            if skip_proj_copies:
                                continue
                            with nc.allow_low_precision("bf16 Q/K"):
                                if m < 2:
                                    nc.vector.tensor_copy(qt_t[m][:, csl], ps[:])
                                else:
                                    nc.vector.tensor_copy(kt_t[m - 2][:, csl],
                                                          ps[:])
                        if skip_v:
                            continue
                        # V natural: x^T tile stationary, Wv moving -> [tokens, vdims]
                        for j in range(4):
                            vp = ps_proj.tile([P, DH], F32, tag="vp",
                                              name=f"vp{n}_{j}")
                            for c in range(NCT):
                                nc.tensor.matmul(
                                    vp[:],
                                    lhsT=xts[c][:, j * P : (j + 1) * P],
                                    rhs=wq_t[c][:, 2 * DH : 3 * DH],
                                    start=(c == 0),
                                    stop=(c == NCT - 1),
                                )
                            if skip_proj_copies:
                                continue
                            # scatter vdims into the per-head 65-col layout
                            va = vaug_t[4 * n + j]
                            with nc.allow_low_precision("bf16 V"):
                                for h in range(NHPC):
                                    nc.vector.tensor_copy(
                                        va[:, h * (D + 1) : h * (D + 1) + D],
                                        vp[:, h * D : (h + 1) * D])

            def emit_stage23():
                with (
                    tc.tile_pool(name="pt", bufs=4) as pt_pool,
                    tc.tile_pool(name="small", bufs=4) as small_pool,
                    tc.tile_pool(name="ysb", bufs=3) as y_pool,
                    tc.tile_pool(name="psst", bufs=2, space="PSUM") as ps_st,
                    # 3 ot buffers: the next head-pair's AV accumulation can
                    # start while the previous pair's normalize still holds one
                    tc.tile_pool(name="psot", bufs=3, space="PSUM") as ps_ot,
                    tc.tile_pool(name="psy", bufs=1, space="PSUM") as ps_y,
                ):
                    pending_out = []

                    def emit_outproj_tile(tt, on_scalar=False, split_k1=False):
                        tsl = slice(tt * P, (tt + 1) * P)
                        for nn in range(2):
                            nsl = slice(nn * CH, (nn + 1) * CH)
                            yp = ps_y.tile([P, CH], F32, tag="y",
                                           name=f"y{tt}_{nn}")
                            nc.tensor.matmul(
                                yp[:],
                                lhsT=ot_t[0][:, tsl],
                                rhs=wot_t[0][:, nsl],
                                start=True,
                                stop=False,
                            )
                            if split_k1:
                                # two K=64 halves: each head-pair half can
                                # start as soon as its normalize lands
                                for hh in range(2):
                                    nc.tensor.matmul(
                                        yp[:],
                                        lhsT=ot_t[1][hh * D : (hh + 1) * D, tsl],
                                        rhs=wot_t[1][hh * D : (hh + 1) * D, nsl],
                                        start=False,
                                        stop=(hh == 1),
                                    )
                            else:
                                nc.tensor.matmul(
                                    yp[:],
                                    lhsT=ot_t[1][:, tsl],
                                    rhs=wot_t[1][:, nsl],
                                    start=False,
                                    stop=True,
                                )
                            ysb = y_pool.tile([P, CH], F32, tag="ysb",
                                              name=f"ysb{tt}_{nn}")
                            # mid-kernel Y copies stay off the Scalar engine
                            # (exp keeps ACT in lockstep with the PE); the
                            # final drain uses the then-idle Scalar engine
                            if on_scalar:
                                nc.scalar.copy(ysb[:], yp[:])
                            else:
                                nc.vector.tensor_copy(ysb[:], yp[:])
                            nc.sync.dma_start(y[tsl, nsl], ysb[:])

                    def emit_st(cq, p, t):
                        st = ps_st.tile([P, 2, CH], F32, tag="st",
                                        name=f"st{cq}_{p}_{t}")
                        tsl = slice(t * P, (t + 1) * P)
                        js = max(0, (t - 4 * cq) * P)
                        qs = slice(cq * CH + js, (cq + 1) * CH)
                        for hh in range(2):
                            nc.tensor.matmul(
                                st[:, hh, js:],
                                lhsT=kt_t[p][hh * D : (hh + 1) * D, tsl],
                                rhs=qt_t[p][hh * D : (hh + 1) * D, qs],
                                start=True,
                                stop=True,
                            )
                        if t >= 4 * cq and not no_mask:
                            nc.vector.tensor_add(
                                st[:, :, js : js + P],
                                st[:, :, js : js + P],
                                mask128[:],
                            )
                        return st, js

                    def norm_steps(cq, p, ots, on_scalar=False):
                        """Yield the normalize chain for (cq, p) as 5 separate
                        steps, so callers can interleave them with the next
                        group's mask adds on the DVE queue."""
                        qsl = slice(cq * CH, (cq + 1) * CH)
                        if no_norm:
                            def all_copies():
                                with nc.allow_low_precision("timing variant"):
                                    for hh in range(2):
                                        nc.vector.tensor_copy(
                                            ot_t[p][hh * D : (hh + 1) * D, qsl],
                                            ots[hh][0:D, :])
                            return [all_copies]
                        cp = nc.scalar.copy if on_scalar else nc.vector.tensor_copy
                        state = {}

                        def phase1(hh):
                            def fn():
                                ot = ots[hh]
                                # denom row straight from PSUM to partition 0
                                # (the custom-DVE approx op needs a partition-0
                                # source); runs parallel to the otu bounce
                                dstg = small_pool.tile([1, CH], F32, tag="ds",
                                                       name=f"ds{cq}_{p}_{hh}")
                                cp(dstg[:], ot[D : D + 1, :])
                                # bounce to SBUF (frees the PSUM slot quickly)
                                otu = small_pool.tile([D, CH], F32, tag="otu",
                                                      name=f"otu{cq}_{p}_{hh}")
                                cp(otu[:], ot[0:D, :])
                                state[hh] = (dstg, otu)
                            return fn

                        def recips():
                            for hh in range(2):
                                dstg, otu = state[hh]
                                recip = small_pool.tile([1, CH], F32, tag="rc",
                                                        name=f"rc{cq}_{p}_{hh}")
                                nc.vector.reciprocal_approx_fast(
                                    recip[:], dstg[:])
                                state[hh] = (recip, otu)

                        def phase2(hh):
                            def fn():
                                recip, otu = state[hh]
                                den = small_pool.tile([D, CH], F32, tag="den",
                                                      name=f"dn{cq}_{p}_{hh}")
                                nc.gpsimd.partition_broadcast(den[:], recip[:])
                                with nc.allow_low_precision("fp32r store"):
                                    nc.vector.tensor_mul(
                                        ot_t[p][hh * D : (hh + 1) * D, qsl],
                                        otu[:],
                                        den[:],
                                    )
                            return fn

                        return [phase1(0), phase1(1), recips,
                                phase2(0), phase2(1)]

                    # one flat score/exp/AV pipeline across every (chunk,
                    # head-pair) boundary — the score matmuls for the next
                    # group fill the PE while the previous group's exp and
                    # normalize complete
                    tiles = [(cq, p, t)
                             for cq in range(NCHUNK)
                             for p in range(2)
                             for t in range(4 * cq + 4)]
                    ots_cur = None
                    norm_q = []     # deferred normalize steps, one per flat step
                    sts = {tiles[0]: emit_st(*tiles[0])}
                    for i, (cq, p, t) in enumerate(tiles):
                        nts = 4 * cq + 4
                        if t == 0:
                            ots_cur = [
                                ps_ot.tile([D + 1, CH], F32, tag="ot",
                                           name=f"ot{cq}_{p}_{hh}")
                                for hh in range(2)
                            ]
                        if i + 1 < len(tiles):
                            sts[tiles[i + 1]] = emit_st(*tiles[i + 1])
                        # one deferred normalize step per flat step: the DVE
                        # queue interleaves them with the diag mask adds
                        # instead of stalling exp behind the whole chain
                        if norm_q:
                            norm_q.pop(0)()
                        st, js = sts.pop((cq, p, t))
                        pt = pt_pool.tile([P, 2, CH], BF16, tag="pt",
                                          name=f"pt{cq}_{p}_{t}")
                        with nc.allow_low_precision("bf16 probs"):
                            nc.scalar.activation(pt[:, :, js:],
                                                 st[:, :, js:], EXP)
                        for hh in range(2):
                            h = 2 * p + hh
                            nc.tensor.matmul(
                                ots_cur[hh][:, js:],
                                lhsT=vaug_t[t][:, h * (D + 1)
                                               : (h + 1) * (D + 1)],
                                rhs=pt[:, hh, js:],
                                start=(t == 0),
                                stop=(t == nts - 1),
                            )
                        # spread previous chunk's out-projection; start late
                        # so the normalize chain never blocks the PE
                        if pending_out and t >= 6:
                            emit_outproj_tile(pending_out.pop(0))
                        if t == nts - 1:
                            norm_q.extend(norm_steps(
                                cq, p, ots_cur,
                                on_scalar=(i == len(tiles) - 1)))
                            if p == 1 and stages >= 3:
                                pending_out.extend(range(4 * cq, 4 * cq + 4))
                    # the last group's normalize runs with no exp work left:
                    # emit the remaining steps immediately on the Scalar engine
                    for fn in norm_q:
                        fn()
                    while pending_out:
                        emit_outproj_tile(pending_out.pop(0), on_scalar=True)

            def emit_dbg_outputs():
                if stages == 1:
                    dbg_srcs = ((wq_t[0], wq_t[1], wq_t[2], wq_t[3])
                                if skip_proj_copies else
                                (qt_t[0], qt_t[1], kt_t[0], kt_t[1]))
                    for i, src_t in enumerate(dbg_srcs):
                        w = min(C, src_t.shape[1])
                        nc.sync.dma_start(y[i * P : (i + 1) * P, 0:w],
                                          src_t[:, 0:w].bitcast(F32))
                    with tc.tile_pool(name="dbg", bufs=2) as dbgp:
                        for tt in range(4):
                            db = dbgp.tile([P, C], F32, tag="db", name=f"db{tt}")
                            nc.gpsimd.memset(db[:], 0.0)
                            for j in range(4):
                                nc.vector.tensor_copy(
                                    db[:, j * 256 : j * 256 + 256],
                                    vaug_t[4 * tt + j][:, 0:256].bitcast(F32))
                            nc.sync.dma_start(y[(4 + tt) * P : (5 + tt) * P, :],
                                              db[:])
                elif stages == 2:
                    for i, src_t in enumerate((ot_t[0], ot_t[1])):
                        nc.sync.dma_start(y[i * P : (i + 1) * P, :],
                                          src_t[:, 0:C].bitcast(F32))

            def emit_body():
                emit_stage1()
                if stages >= 2:
                    emit_stage23()
                emit_dbg_outputs()

            if loop_reps is None:
                emit_body()
            else:
                with tc.For_i(0, loop_reps, 1):
                    emit_body()

    nc.finalize()
    return nc


_NC_CACHE = None


def get_nc():
    global _NC_CACHE
    if _NC_CACHE is None:
        _NC_CACHE = build_nc()
    return _NC_CACHE


def make_in_maps(x, Wq, Wk, Wv, Wo):
    import ml_dtypes

    bf16 = ml_dtypes.bfloat16
    scale = 1.0 / np.sqrt(np.float32(C))
    in_maps = []
    for core in range(8):
        b, hg = core // 4, core % 4
        hsl = slice(hg * NHPC, (hg + 1) * NHPC)
        xT = np.ascontiguousarray(x[b].T).astype(bf16)
        wq = (Wq[hsl] * scale).transpose(1, 0, 2).reshape(C, DH)
        wk = Wk[hsl].transpose(1, 0, 2).reshape(C, DH)
        wv = Wv[hsl].transpose(1, 0, 2).reshape(C, DH)
        wqkv = np.ascontiguousarray(
            np.concatenate([wq, wk, wv], axis=1, dtype=np.float32)).astype(bf16)
        wot = np.ascontiguousarray(Wo[:, hg * DH : (hg + 1) * DH].T).astype(bf16)
        in_maps.append({
            "xT": xT,
            "Wqkv": wqkv,
            "WoT": wot,
        })
    return in_maps


def gather(results, bo):
    out = np.zeros((B, T, C), dtype=np.float32)
    for core in range(8):
        out[core // 4] += results[core]["Y"]
    out += bo.astype(np.float32)
    return out


def kernel(x, Wq, Wk, Wv, Wo, bo, **run_kwargs):
    x = np.asarray(x, dtype=np.float32)
    Wq = np.asarray(Wq, dtype=np.float32)
    Wk = np.asarray(Wk, dtype=np.float32)
    Wv = np.asarray(Wv, dtype=np.float32)
    Wo = np.asarray(Wo, dtype=np.float32)
    bo = np.asarray(bo, dtype=np.float32)
    nc = get_nc()
    in_maps = make_in_maps(x, Wq, Wk, Wv, Wo)
    res = run_bass_kernel_spmd(nc, in_maps, core_ids=list(range(8)), **run_kwargs)
    out = gather(res.results, bo)
    if run_kwargs:
        return out, res
    return out

